# revision 20
# baseline (speedup 1.0000x reference)
"""Multi-head attention (B=2, S=2048, D=1024, H=16) on 8 TRN2 NeuronCores.

Sharding: (batch, head-group) SPMD. Core c handles batch b = c//4 and local
heads [4*(c%4), 4*(c%4)+4). Each core computes its 4 heads' attention plus the
partial o-projection (row-parallel over the head dimension); the host sums the
4 partial outputs per batch and adds b_o.

v5 structure:
  - x is DMA'd once into 8 resident [128,2048] chunk tiles feeding every
    projection consumer. Weights repacked [K|Q|V] so the K slices stream
    first; DMA issues are fused (the sync queue serializes issues at
    ~0.65us each). Mask is fp8 (0/1 exact) -> half the DMA bytes.
  - warm-up matmuls ramp the PE p-state while the first x chunks land.
  - pre-phase (own PSUM pool): joint k-outer pass producing K^T cols 0:1536
    (both pairs) + Q-q0, then V seq-tiles 0,1. Remaining projections (K-tail
    cols 1536:2048, V st 2-15, Q quarters 1-3) are woven into phase 2 as
    short psa-rotation jobs.
  - phase 2: S/E/M/P pipeline as before, but ctx PSUM (cq) is split into two
    per-pair [128,1024] tiles in a bufs=2 rotation ("cqp" tag) that ALSO
    carries the boundary PSUM users (1/den rank-1 broadcasts, o_proj opp
    tiles) in dependency order -- the tq/"psa" rotation stays reserved for
    the exp stream, so qb boundaries no longer stall it.
  - softmax denominators: den row (partition 64) -> bf16 cast in-lane ->
    rank-1 PE broadcast (ones[1,64] stationary) into PSUM -> reciprocal on
    the partition-0-aligned block. No DMA hop, no gpsimd.
"""
import os
import sys

if "/opt/trn_rl_repo" not in sys.path:
    sys.path.insert(0, "/opt/trn_rl_repo")
os.environ.setdefault("JAX_PLATFORMS", "axon,cpu")

from collections import defaultdict
from contextlib import ExitStack

import ml_dtypes
import numpy as np

import concourse.bass as bass
import concourse.tile as tile
from concourse import bacc, library_config, mybir
from concourse.bass_utils import run_bass_kernel_spmd

F32 = mybir.dt.float32
BF16 = mybir.dt.bfloat16
FP8 = mybir.dt.float8e4
EXP = mybir.ActivationFunctionType.Exp

B, S, D = 2, 2048, 1024
H, HD = 16, 64
HL = 4            # local heads per core
CH = HL * HD      # 256 local channels
N_CORES = 8
KC = D // 128     # 8 contraction chunks for the projections
NQB = S // 512    # 4 q blocks
NKT = S // 128    # 16 k tiles
NIT = NQB * NKT * 2   # 128 pipeline iterations
W3 = 3 * CH
PT_BUFS = 16
PGAP = 2          # extra P-lag added per qb boundary
NWARM = 8         # PE p-state warm-up matmuls

_CACHE = {}


def _build_nc():
    nc = bacc.Bacc("TRN2", target_bir_lowering=False)
    xT_d = nc.declare_dram_parameter("xT", [D, S], BF16, isOutput=False)
    mk_d = nc.declare_dram_parameter("maskT", [S, S], FP8, isOutput=False)
    wqkvT_d = nc.declare_dram_parameter("wqkvT", [D, 3 * CH], BF16, isOutput=False)
    woT_d = nc.declare_dram_parameter("woT", [CH, D], BF16, isOutput=False)
    yT_d = nc.declare_dram_parameter("yT", [D, S], BF16, isOutput=True)

    with tile.TileContext(nc) as tc, ExitStack() as ctx:
        nc.gpsimd.load_library(library_config.attn)
        const = ctx.enter_context(tc.tile_pool(name="const", bufs=1))

        # ---- resident tensors ----
        mkq = [const.tile([128, 4 * S], FP8, name=f"mkq{j}") for j in range(4)]
        wo_all = const.tile([128, 2 * D], BF16, name="wo")
        qt = [const.tile([128, S], BF16, name=f"qt{i}") for i in range(2)]
        kt_sb = [const.tile([128, S], BF16, name=f"kt{i}") for i in range(2)]
        v_sb = [const.tile([128, HL * 65], BF16, name=f"v{i}") for i in range(NKT)]
        wsb = const.tile([128, KC * W3], BF16, name="w")
        xk = [const.tile([128, S], BF16, name=f"xk{k}") for k in range(KC)]
        ones64 = const.tile([65, 64], BF16, name="ones64")
        nc.gpsimd.memset(ones64[:], 1.0)
        for st in range(NKT):
            nc.gpsimd.memset(
                v_sb[st].rearrange("p (h c) -> p h c", h=HL)[:, :, 64:65], 1.0
            )

        # weight column offsets in the [K|Q|V]-packed wsb
        WK, WQ, WV = 0, CH, 2 * CH

        with tc.tile_pool(name="work", bufs=1) as work:
            # ---- DMA preamble (consumption order; issues are serialized on
            # the in-order sync queue at ~0.65us each, so they are fused) ----
            for kp in range(4):
                k0 = 2 * kp
                nc.sync.dma_start(
                    wsb[:].rearrange("p (k w) -> p k w", k=KC)[
                        :, k0:k0 + 2, WK:WK + CH],
                    wqkvT_d[k0 * 128:(k0 + 2) * 128, 0:CH].rearrange(
                        "(k p) w -> p k w", k=2),
                )
                nc.sync.dma_start(xk[k0][:], xT_d[k0 * 128:(k0 + 1) * 128, :])
                nc.sync.dma_start(xk[k0 + 1][:],
                                  xT_d[(k0 + 1) * 128:(k0 + 2) * 128, :])
            for kp in range(4):
                k0 = 2 * kp
                nc.sync.dma_start(
                    wsb[:].rearrange("p (k w) -> p k w", k=KC)[
                        :, k0:k0 + 2, WQ:WQ + 2 * CH],
                    wqkvT_d[k0 * 128:(k0 + 2) * 128, CH:3 * CH].rearrange(
                        "(k p) w -> p k w", k=2),
                )
            for j in range(4):
                nc.sync.dma_start(
                    mkq[j][:].rearrange("p (t c) -> p t c", t=4),
                    mk_d[j * 512:(j + 1) * 512, :].rearrange(
                        "(t p) c -> p t c", t=4),
                )
            nc.sync.dma_start(
                wo_all[:].rearrange("p (j w) -> p j w", j=2),
                woT_d[:].rearrange("(j p) w -> p j w", j=2),
            )

            def v_job(pool, st, tag, bufs):
                vp = pool.tile([128, CH], F32, name="vp", tag=tag, bufs=bufs)
                for k in range(KC):
                    nc.tensor.matmul(
                        vp[:],
                        xk[k][:, st * 128:(st + 1) * 128],
                        wsb[:, k * W3 + WV:k * W3 + WV + CH],
                        start=(k == 0), stop=(k == KC - 1),
                    )
                nc.vector.tensor_copy(
                    v_sb[st].rearrange("p (h c) -> p h c", h=HL)[:, :, 0:64],
                    vp.rearrange("p (h c) -> p h c", h=HL),
                )

            # ---- pre-phase: warm-up, joint K(0:1536)+Q-q0 pass, V st0,1 ----
            with tc.tile_pool(name="kq", bufs=1, space="PSUM") as kq:
                warm = kq.tile([128, 1024], F32, name="warm", tag="kc", bufs=1)
                for _ in range(NWARM):
                    nc.tensor.matmul(warm[:, 0:256], wsb[:, 0:128],
                                     wsb[:, 0:256], start=True, stop=True)

                ktP = [kq.tile([128, 1536], F32, name=f"ktP{p}", tag=t, bufs=1)
                       for p, t in ((0, "ka"), (1, "kb"))]
                qP = kq.tile([128, 1024], F32, name="qP", tag="kc", bufs=1)
                for k in range(KC):
                    for p in range(2):
                        nc.tensor.matmul(
                            qP[:, p * 512:(p + 1) * 512],
                            wsb[:, k * W3 + WQ + p * 128:
                                k * W3 + WQ + (p + 1) * 128],
                            xk[k][:, 0:512],
                            start=(k == 0), stop=(k == KC - 1),
                        )
                        wst = wsb[:, k * W3 + WK + p * 128:
                                  k * W3 + WK + (p + 1) * 128]
                        for s3 in range(3):
                            nc.tensor.matmul(
                                ktP[p][:, 512 * s3:512 * (s3 + 1)],
                                wst, xk[k][:, 512 * s3:512 * (s3 + 1)],
                                start=(k == 0), stop=(k == KC - 1),
                            )
                # evacuations split across DVE and ACT so the slots the V-pass
                # and K-tail need free up in parallel
                nc.vector.tensor_copy(kt_sb[0][:, 0:1536], ktP[0][:, 0:1536])
                nc.scalar.copy(kt_sb[1][:, 0:1536], ktP[1][:, 0:1536])
                for p in range(2):
                    nc.vector.tensor_copy(qt[p][:, 0:512],
                                          qP[:, p * 512:(p + 1) * 512])
                # V seq-tiles 0,1 then K-tail (cols 1536:2048) then V 2..15,
                # pipelined through the freed ka/kb/kc slots
                v_job(kq, 0, "ka", 1)
                v_job(kq, 1, "kb", 1)
                for p in range(2):
                    ktT = kq.tile([128, 512], F32, name=f"ktT{p}", tag="kc",
                                  bufs=1)
                    for k in range(KC):
                        nc.tensor.matmul(
                            ktT[:],
                            wsb[:, k * W3 + WK + p * 128:
                                k * W3 + WK + (p + 1) * 128],
                            xk[k][:, 1536:2048],
                            start=(k == 0), stop=(k == KC - 1),
                        )
                    nc.vector.tensor_copy(kt_sb[p][:, 1536:2048], ktT[:])
                for st in range(2, NKT):
                    v_job(kq, st, "ka" if st % 2 == 0 else "kb", 1)

            # ---- phase 2 ----
            psum = ctx.enter_context(tc.tile_pool(name="psum", bufs=1, space="PSUM"))

            def it_decode(i):
                return i // 32, (i // 2) % 16, i % 2   # qb, ktile, pair

            lp = {qb: (qb * 32 + 31) + 4 + PGAP * qb for qb in range(NQB)}
            floor = {0: 0}
            for qb in range(1, NQB):
                # first P of qb must be emitted after qb-1's last opp tile:
                # the cqp rotation order is [cqA, cqB, rbp0, rbp1, opp0..3]
                floor[qb] = lp[qb - 1] + 9

            sched = defaultdict(list)
            sched[21].append(("QJ", 1, 0))
            sched[23].append(("QJ", 1, 1))
            sched[55].append(("QJ", 2, 0))
            sched[57].append(("QJ", 2, 1))
            sched[87].append(("QJ", 3, 0))
            sched[89].append(("QJ", 3, 1))
            for i in range(NIT):
                qb = i // 32
                sched[i].append(("S", i))
                sched[i + 1].append(("E", i))
                sched[i + 2].append(("M", i))
                # late emission + 2-per-group catch-up after the boundary
                # P-pause, so queued P's never block later S's in the PE queue
                pg = max(i + 4 + PGAP * qb, floor[qb] + (i - 32 * qb) // 2)
                sched[pg].append(("P", i))
            for qb in range(NQB):
                g = lp[qb]
                for c in range(4):
                    sched[g + 1].append(("CPc", qb, c))
                sched[g + 2].append(("R2", qb, 0))
                sched[g + 3].append(("R2", qb, 1))
                sched[g + 3].append(("CN", qb, 0))
                sched[g + 3].append(("CN", qb, 1))
                sched[g + 4].append(("CN", qb, 2))
                sched[g + 4].append(("CN", qb, 3))
                for g4 in range(4):
                    sched[g + 5 + g4].append(("O", qb, g4))
            ngroups = max(sched) + 1

            tq_t, ex_t, pt_t, cq_t, cn_t = {}, {}, {}, {}, {}
            for g in range(ngroups):
                for op in sched[g]:
                    kind = op[0]
                    if kind == "S":
                        i = op[1]
                        qb, ktile, pair = it_decode(i)
                        tq = psum.tile([128, 1024], F32, name="psa", tag="psa", bufs=2)
                        for hh in range(2):
                            nc.tensor.matmul(
                                tq[:, hh * 512:(hh + 1) * 512],
                                kt_sb[pair][hh * 64:(hh + 1) * 64,
                                            ktile * 128:(ktile + 1) * 128],
                                qt[pair][hh * 64:(hh + 1) * 64,
                                         qb * 512:(qb + 1) * 512],
                                start=True, stop=True,
                            )
                        tq_t[i] = tq
                    elif kind == "E":
                        i = op[1]
                        ex = work.tile([128, 1024], BF16, name="expq", tag="expq", bufs=3)
                        nc.scalar.activation(ex[:], tq_t.pop(i)[:], EXP)
                        ex_t[i] = ex
                    elif kind == "M":
                        i = op[1]
                        qb, ktile, pair = it_decode(i)
                        ex = ex_t.pop(i)
                        pt = work.tile([128, 1024], BF16, name="pt", tag="pt",
                                       bufs=PT_BUFS)
                        mq = mkq[ktile // 4]
                        mof = (ktile % 4) * S + qb * 512
                        for hh in range(2):
                            nc.vector.tensor_mul(
                                pt[:, hh * 512:(hh + 1) * 512],
                                ex[:, hh * 512:(hh + 1) * 512],
                                mq[:, mof:mof + 512],
                            )
                        pt_t[i] = pt
                    elif kind == "P":
                        i = op[1]
                        qb, ktile, pair = it_decode(i)
                        if i % 32 in (0, 1):
                            cq_t[(qb, pair)] = psum.tile(
                                [128, 1024], F32, name=f"cq{pair}",
                                tag="cqp", bufs=2)
                        cq = cq_t[(qb, pair)]
                        pt = pt_t.pop(i)
                        for hh in range(2):
                            h = pair * 2 + hh
                            nc.tensor.matmul(
                                cq[0:65, hh * 512:(hh + 1) * 512],
                                v_sb[ktile][:, h * 65:h * 65 + 65],
                                pt[:, hh * 512:(hh + 1) * 512],
                                start=(ktile == 0), stop=(ktile == NKT - 1),
                            )
                    elif kind == "VJ":
                        v_job(psum, op[1], "psa", 2)
                    elif kind == "KT":
                        p = op[1]
                        ps = psum.tile([128, 512], F32, name="psk", tag="psa", bufs=2)
                        for k in range(KC):
                            nc.tensor.matmul(
                                ps[:],
                                wsb[:, k * W3 + WK + p * 128:
                                    k * W3 + WK + (p + 1) * 128],
                                xk[k][:, 1536:2048],
                                start=(k == 0), stop=(k == KC - 1),
                            )
                        nc.vector.tensor_copy(kt_sb[p][:, 1536:2048], ps[:])
                    elif kind == "QJ":
                        q, p = op[1], op[2]
                        ps = psum.tile([128, 512], F32, name="psq", tag="psa", bufs=2)
                        for k in range(KC):
                            nc.tensor.matmul(
                                ps[:],
                                wsb[:, k * W3 + WQ + p * 128:
                                    k * W3 + WQ + (p + 1) * 128],
                                xk[k][:, q * 512:(q + 1) * 512],
                                start=(k == 0), stop=(k == KC - 1),
                            )
                        nc.vector.tensor_copy(qt[p][:, q * 512:(q + 1) * 512], ps[:])
                    elif kind == "CPc":
                        qb, c = op[1], op[2]
                        if c == 0:
                            _CACHE.setdefault("cqs_t", {})[qb] = work.tile(
                                [65, 2048], F32, name="cqs", tag="cqs", bufs=1)
                            rcb = work.tile([65, 2048], BF16, name="rcb",
                                            tag="rcb", bufs=1)
                            cn2 = work.tile([128, 1024], BF16, name="cn2",
                                            tag="cn2", bufs=1)
                            cno = work.tile([64, 1024], BF16, name="cno",
                                            tag="cno", bufs=1)
                            _CACHE.setdefault("rb_t", {})[qb] = (rcb, cn2, cno)
                        pair, hh = c // 2, c % 2
                        cqs = _CACHE["cqs_t"][qb]
                        cq = cq_t[(qb, pair)]
                        dst = cqs[:, c * 512:(c + 1) * 512]
                        src = cq[0:65, hh * 512:(hh + 1) * 512]
                        if qb < NQB - 1:
                            nc.vector.tensor_copy(dst, src)
                        else:
                            nc.scalar.copy(dst, src)
                        if c == 3:
                            cq_t.pop((qb, 0))
                            cq_t.pop((qb, 1))
                    elif kind == "R2":
                        qb, half = op[1], op[2]
                        rcb, cn2, cno = _CACHE["rb_t"][qb]
                        cqs = _CACHE["cqs_t"][qb]
                        sl = slice(half * 1024, (half + 1) * 1024)
                        nc.vector.tensor_copy(rcb[64:65, sl], cqs[64:65, sl])
                        rbp = psum.tile([64, 1024], F32, name="rbp", tag="cqp",
                                        bufs=2)
                        for c2 in range(2):
                            nc.tensor.matmul(
                                rbp[:, c2 * 512:(c2 + 1) * 512],
                                ones64[64:65, 0:64],
                                rcb[64:65, (half * 2 + c2) * 512:
                                    (half * 2 + c2 + 1) * 512],
                                start=True, stop=True,
                            )
                        rb32 = work.tile([64, 1024], F32, name="rb32",
                                         tag="rb32", bufs=2)
                        nc.vector.reciprocal_approx_fast(rb32[:], rbp[:])
                        _CACHE.setdefault("rbp_t", {})[(qb, half)] = rb32
                    elif kind == "CN":
                        qb, c = op[1], op[2]
                        rcb, cn2, cno = _CACHE["rb_t"][qb]
                        cqs = _CACHE["cqs_t"][qb]
                        rb32 = _CACHE["rbp_t"][(qb, c // 2)]
                        j = c // 2
                        src = cqs[0:64, c * 512:(c + 1) * 512]
                        rbc = rb32[:, (c % 2) * 512:(c % 2 + 1) * 512]
                        if c % 2 == 0:
                            nc.vector.tensor_mul(
                                cn2[0:64, j * 512:(j + 1) * 512], src, rbc)
                        else:
                            nc.vector.tensor_mul(
                                cno[:, j * 512:(j + 1) * 512], src, rbc)
                            nc.sync.dma_start(
                                cn2[64:128, j * 512:(j + 1) * 512],
                                cno[:, j * 512:(j + 1) * 512])
                        if c == 3:
                            cn_t[qb] = cn2
                            _CACHE["rb_t"].pop(qb)
                            _CACHE["cqs_t"].pop(qb)
                            _CACHE["rbp_t"].pop((qb, 0))
                            _CACHE["rbp_t"].pop((qb, 1))
                    elif kind == "O":
                        qb, g4 = op[1], op[2]
                        cn2 = cn_t[qb]
                        opp = psum.tile([128, 1024], F32, name="opp", tag="cqp",
                                        bufs=2)
                        for ot_l in range(2):
                            ot = 2 * g4 + ot_l
                            for j in range(2):
                                nc.tensor.matmul(
                                    opp[:, ot_l * 512:(ot_l + 1) * 512],
                                    wo_all[:, j * D + ot * 128:
                                           j * D + (ot + 1) * 128],
                                    cn2[:, j * 512:(j + 1) * 512],
                                    start=(j == 0), stop=(j == 1),
                                )
                        ysb = work.tile([128, 1024], BF16, name="ysb", tag="ysb", bufs=2)
                        if qb == NQB - 1 and g4 % 2 == 0:
                            nc.scalar.copy(ysb[:], opp[:])
                        else:
                            nc.vector.tensor_copy(ysb[:], opp[:])
                        nc.sync.dma_start(
                            yT_d[g4 * 256:(g4 + 1) * 256,
                                 qb * 512:(qb + 1) * 512].rearrange(
                                     "(o r) c -> r o c", o=2),
                            ysb.rearrange("r (o c) -> r o c", o=2),
                        )
                        if g4 == 3:
                            cn_t.pop(qb)
    nc.compile()
    return nc


def _get_nc():
    if "nc" not in _CACHE:
        _CACHE["nc"] = _build_nc()
    return _CACHE["nc"]


def kernel(x, mask, w_qkv, b_qkv, w_o, b_o):
    x = np.asarray(x, dtype=np.float32)
    mask = np.asarray(mask)
    w_qkv = np.asarray(w_qkv, dtype=np.float32)
    b_qkv = np.asarray(b_qkv, dtype=np.float32)
    w_o = np.asarray(w_o, dtype=np.float32)
    b_o = np.asarray(b_o, dtype=np.float32)
    assert not b_qkv.any(), "kernel specialized for zero qkv bias"

    scale = np.float32(1.0 / np.sqrt(HD))
    maskT = np.ascontiguousarray(mask.reshape(S, S).T).astype(
        ml_dtypes.float8_e4m3)

    w3 = w_qkv.reshape(H, 3, HD, D)  # [head, (q,k,v), hd, D]
    in_maps = []
    for c in range(N_CORES):
        b = c // 4
        h0 = (c % 4) * HL
        heads = list(range(h0, h0 + HL))
        wq = w3[heads, 0].reshape(CH, D) * scale
        wk = w3[heads, 1].reshape(CH, D)
        wv = w3[heads, 2].reshape(CH, D)
        # [K | Q | V] column packing
        wqkv = np.concatenate([wk.T, wq.T, wv.T], axis=1)  # [D, 3CH]
        wo_cols = np.concatenate([w_o[:, h * HD:(h + 1) * HD] for h in heads], axis=1)
        in_maps.append({
            "xT": np.ascontiguousarray(x[b].T).astype(ml_dtypes.bfloat16),
            "maskT": maskT,
            "wqkvT": np.ascontiguousarray(wqkv).astype(ml_dtypes.bfloat16),
            "woT": np.ascontiguousarray(wo_cols.T).astype(ml_dtypes.bfloat16),
        })

    nc = _get_nc()
    trace = bool(int(os.environ.get("MHA_TRACE", "0")))
    res = run_bass_kernel_spmd(nc, in_maps, core_ids=list(range(N_CORES)),
                               trace=trace)
    _CACHE["last_results"] = res

    y = np.zeros((B, S, D), dtype=np.float32)
    for c in range(N_CORES):
        y[c // 4] += np.asarray(res.results[c]["yT"], dtype=np.float32).T
    y += b_o
    return y


# revision 25
# speedup vs baseline: 1.0375x; 1.0375x over previous
"""Multi-head attention (B=2, S=2048, D=1024, H=16) on 8 TRN2 NeuronCores.

Sharding: (batch, head-group) SPMD. Core c handles batch b = c//4 and local
heads [4*(c%4), 4*(c%4)+4). Each core computes its 4 heads' attention plus the
partial o-projection (row-parallel over the head dimension); the host sums the
4 partial outputs per batch and adds b_o.

v5 structure:
  - x is DMA'd once into 8 resident [128,2048] chunk tiles feeding every
    projection consumer. Weights repacked [K|Q|V] so the K slices stream
    first; DMA issues are fused (the sync queue serializes issues at
    ~0.65us each). Mask is fp8 (0/1 exact) -> half the DMA bytes.
  - warm-up matmuls ramp the PE p-state while the first x chunks land.
  - pre-phase (own PSUM pool): joint k-outer pass producing K^T cols 0:1536
    (both pairs) + Q-q0, then V seq-tiles 0,1. Remaining projections (K-tail
    cols 1536:2048, V st 2-15, Q quarters 1-3) are woven into phase 2 as
    short psa-rotation jobs.
  - phase 2: S/E/M/P pipeline as before, but ctx PSUM (cq) is split into two
    per-pair [128,1024] tiles in a bufs=2 rotation ("cqp" tag) that ALSO
    carries the boundary PSUM users (1/den rank-1 broadcasts, o_proj opp
    tiles) in dependency order -- the tq/"psa" rotation stays reserved for
    the exp stream, so qb boundaries no longer stall it.
  - softmax denominators: den row (partition 64) -> bf16 cast in-lane ->
    rank-1 PE broadcast (ones[1,64] stationary) into PSUM -> reciprocal on
    the partition-0-aligned block. No DMA hop, no gpsimd.
"""
import os
import sys

if "/opt/trn_rl_repo" not in sys.path:
    sys.path.insert(0, "/opt/trn_rl_repo")
os.environ.setdefault("JAX_PLATFORMS", "axon,cpu")

from collections import defaultdict
from contextlib import ExitStack

import ml_dtypes
import numpy as np

import concourse.bass as bass
import concourse.tile as tile
from concourse import bacc, library_config, mybir
from concourse.bass_utils import run_bass_kernel_spmd

F32 = mybir.dt.float32
BF16 = mybir.dt.bfloat16
FP8 = mybir.dt.float8e4
EXP = mybir.ActivationFunctionType.Exp

B, S, D = 2, 2048, 1024
H, HD = 16, 64
HL = 4            # local heads per core
CH = HL * HD      # 256 local channels
N_CORES = 8
KC = D // 128     # 8 contraction chunks for the projections
NQB = S // 512    # 4 q blocks
NKT = S // 128    # 16 k tiles
NIT = NQB * NKT * 2   # 128 pipeline iterations
W3 = 3 * CH
PT_BUFS = 16
PGAP = 2          # extra P-lag added per qb boundary
NWARM = 8         # PE p-state warm-up matmuls

_CACHE = {}


def _build_nc():
    nc = bacc.Bacc("TRN2", target_bir_lowering=False)
    xT_d = nc.declare_dram_parameter("xT", [D, S], BF16, isOutput=False)
    mk_d = nc.declare_dram_parameter("maskT", [S, S], BF16, isOutput=False)
    wqkvT_d = nc.declare_dram_parameter("wqkvT", [D, 3 * CH], BF16, isOutput=False)
    woT_d = nc.declare_dram_parameter("woT", [CH, D], BF16, isOutput=False)
    yT_d = nc.declare_dram_parameter("yT", [D, S], BF16, isOutput=True)

    with tile.TileContext(nc) as tc, ExitStack() as ctx:
        nc.gpsimd.load_library(library_config.attn)
        const = ctx.enter_context(tc.tile_pool(name="const", bufs=1))

        # ---- resident tensors ----
        mkq = [const.tile([128, 4 * S], BF16, name=f"mkq{j}") for j in range(4)]
        wo_all = const.tile([128, 2 * D], BF16, name="wo")
        qt = [const.tile([128, S], BF16, name=f"qt{i}") for i in range(2)]
        kt_sb = [const.tile([128, S], BF16, name=f"kt{i}") for i in range(2)]
        v_sb = [const.tile([128, HL * 65], BF16, name=f"v{i}") for i in range(NKT)]
        wsb = const.tile([128, KC * W3], BF16, name="w")
        xk = [const.tile([128, S], BF16, name=f"xk{k}") for k in range(KC)]
        ones64 = const.tile([65, 64], BF16, name="ones64")
        nc.gpsimd.memset(ones64[:], 1.0)
        for st in range(NKT):
            nc.gpsimd.memset(
                v_sb[st].rearrange("p (h c) -> p h c", h=HL)[:, :, 64:65], 1.0
            )

        # weight column offsets in the [K|Q|V]-packed wsb
        WK, WQ, WV = 0, CH, 2 * CH

        with tc.tile_pool(name="work", bufs=1) as work:
            # ---- DMA preamble (consumption order; issues are serialized on
            # the in-order sync queue at ~0.65us each, so they are fused) ----
            for kp in range(4):
                k0 = 2 * kp
                nc.sync.dma_start(
                    wsb[:].rearrange("p (k w) -> p k w", k=KC)[
                        :, k0:k0 + 2, WK:WK + CH],
                    wqkvT_d[k0 * 128:(k0 + 2) * 128, 0:CH].rearrange(
                        "(k p) w -> p k w", k=2),
                )
                nc.sync.dma_start(xk[k0][:], xT_d[k0 * 128:(k0 + 1) * 128, :])
                nc.sync.dma_start(xk[k0 + 1][:],
                                  xT_d[(k0 + 1) * 128:(k0 + 2) * 128, :])
            for kp in range(4):
                k0 = 2 * kp
                nc.sync.dma_start(
                    wsb[:].rearrange("p (k w) -> p k w", k=KC)[
                        :, k0:k0 + 2, WQ:WQ + 2 * CH],
                    wqkvT_d[k0 * 128:(k0 + 2) * 128, CH:3 * CH].rearrange(
                        "(k p) w -> p k w", k=2),
                )
            for j in range(4):
                nc.sync.dma_start(
                    mkq[j][:].rearrange("p (t c) -> p t c", t=4),
                    mk_d[j * 512:(j + 1) * 512, :].rearrange(
                        "(t p) c -> p t c", t=4),
                )
            nc.sync.dma_start(
                wo_all[:].rearrange("p (j w) -> p j w", j=2),
                woT_d[:].rearrange("(j p) w -> p j w", j=2),
            )

            def v_job(pool, st, tag, bufs):
                vp = pool.tile([128, CH], F32, name="vp", tag=tag, bufs=bufs)
                for k in range(KC):
                    nc.tensor.matmul(
                        vp[:],
                        xk[k][:, st * 128:(st + 1) * 128],
                        wsb[:, k * W3 + WV:k * W3 + WV + CH],
                        start=(k == 0), stop=(k == KC - 1),
                    )
                nc.vector.tensor_copy(
                    v_sb[st].rearrange("p (h c) -> p h c", h=HL)[:, :, 0:64],
                    vp.rearrange("p (h c) -> p h c", h=HL),
                )

            # ---- pre-phase: warm-up, full-K joint pass (DMA-paced), Q-q0,
            # V-pass — all pipelined through the two 4-bank kq slots ----
            with tc.tile_pool(name="kq", bufs=1, space="PSUM") as kq:
                warm = kq.tile([128, 1024], F32, name="warm", tag="ka", bufs=1)
                for _ in range(NWARM):
                    nc.tensor.matmul(warm[:, 0:256], wsb[:, 0:128],
                                     wsb[:, 0:256], start=True, stop=True)

                ktP = [kq.tile([128, S], F32, name=f"ktP{p}", tag=t, bufs=1)
                       for p, t in ((0, "ka"), (1, "kb"))]
                for k in range(KC):
                    for p in range(2):
                        wst = wsb[:, k * W3 + WK + p * 128:
                                  k * W3 + WK + (p + 1) * 128]
                        for s4 in range(4):
                            nc.tensor.matmul(
                                ktP[p][:, 512 * s4:512 * (s4 + 1)],
                                wst, xk[k][:, 512 * s4:512 * (s4 + 1)],
                                start=(k == 0), stop=(k == KC - 1),
                            )
                # evacuations split across DVE and ACT so both slots free in
                # parallel
                nc.vector.tensor_copy(kt_sb[0][:], ktP[0][:])
                nc.scalar.copy(kt_sb[1][:], ktP[1][:])
                # Q-q0 (k-inner, rides slot ka), then the V-pass alternating
                # slots
                qP = kq.tile([128, 1024], F32, name="qP", tag="ka", bufs=1)
                for k in range(KC):
                    for p in range(2):
                        nc.tensor.matmul(
                            qP[:, p * 512:(p + 1) * 512],
                            wsb[:, k * W3 + WQ + p * 128:
                                k * W3 + WQ + (p + 1) * 128],
                            xk[k][:, 0:512],
                            start=(k == 0), stop=(k == KC - 1),
                        )
                for p in range(2):
                    nc.vector.tensor_copy(qt[p][:, 0:512],
                                          qP[:, p * 512:(p + 1) * 512])
                v_job(kq, 0, "kb", 1)
                for st in range(1, NKT):
                    v_job(kq, st, "ka" if st % 2 == 1 else "kb", 1)

            # ---- phase 2 ----
            psum = ctx.enter_context(tc.tile_pool(name="psum", bufs=1, space="PSUM"))

            def it_decode(i):
                return i // 32, (i // 2) % 16, i % 2   # qb, ktile, pair

            lp = {qb: (qb * 32 + 31) + 4 + PGAP * qb for qb in range(NQB)}
            floor = {0: 0}
            for qb in range(1, NQB):
                # first P of qb must be emitted after qb-1's last opp tile:
                # the cqp rotation order is [cqA, cqB, rbp0, rbp1, opp0..3]
                floor[qb] = lp[qb - 1] + 9

            sched = defaultdict(list)
            sched[21].append(("QJ", 1, 0))
            sched[23].append(("QJ", 1, 1))
            sched[55].append(("QJ", 2, 0))
            sched[57].append(("QJ", 2, 1))
            sched[87].append(("QJ", 3, 0))
            sched[89].append(("QJ", 3, 1))
            for i in range(NIT):
                qb = i // 32
                sched[i].append(("S", i))
                sched[i + 1].append(("E", i))
                sched[i + 2].append(("M", i))
                # late emission + 2-per-group catch-up after the boundary
                # P-pause, so queued P's never block later S's in the PE queue
                pg = max(i + 4 + PGAP * qb, floor[qb] + (i - 32 * qb) // 2)
                sched[pg].append(("P", i))
            for qb in range(NQB):
                g = lp[qb]
                for c in range(4):
                    sched[g + 1].append(("CPc", qb, c))
                sched[g + 2].append(("R2", qb, 0))
                sched[g + 3].append(("R2", qb, 1))
                sched[g + 3].append(("CN", qb, 0))
                sched[g + 3].append(("CN", qb, 1))
                sched[g + 4].append(("CN", qb, 2))
                sched[g + 4].append(("CN", qb, 3))
                for g4 in range(4):
                    sched[g + 5 + g4].append(("O", qb, g4))
            ngroups = max(sched) + 1

            # within a group: S/E/M first, then jobs, then P, then boundary —
            # a dep-stalled P must never sit ahead of an S in the PE queue
            PRIO = {"S": 0, "E": 1, "M": 2, "QJ": 3, "P": 4, "CPc": 5,
                    "R2": 6, "CN": 7, "O": 8}
            tq_t, ex_t, pt_t, cq_t, cn_t = {}, {}, {}, {}, {}
            for g in range(ngroups):
                for op in sorted(sched[g], key=lambda o: PRIO[o[0]]):
                    kind = op[0]
                    if kind == "S":
                        i = op[1]
                        qb, ktile, pair = it_decode(i)
                        tq = psum.tile([128, 1024], F32, name="psa", tag="psa", bufs=2)
                        for hh in range(2):
                            nc.tensor.matmul(
                                tq[:, hh * 512:(hh + 1) * 512],
                                kt_sb[pair][hh * 64:(hh + 1) * 64,
                                            ktile * 128:(ktile + 1) * 128],
                                qt[pair][hh * 64:(hh + 1) * 64,
                                         qb * 512:(qb + 1) * 512],
                                start=True, stop=True,
                            )
                        tq_t[i] = tq
                    elif kind == "E":
                        i = op[1]
                        ex = work.tile([128, 1024], BF16, name="expq", tag="expq", bufs=3)
                        nc.scalar.activation(ex[:], tq_t.pop(i)[:], EXP)
                        ex_t[i] = ex
                    elif kind == "M":
                        i = op[1]
                        qb, ktile, pair = it_decode(i)
                        ex = ex_t.pop(i)
                        pt = work.tile([128, 1024], BF16, name="pt", tag="pt",
                                       bufs=PT_BUFS)
                        mq = mkq[ktile // 4]
                        mof = (ktile % 4) * S + qb * 512
                        # the first iterations after a qb boundary run on
                        # gpsimd: the DVE is busy with the boundary chain
                        eng = nc.gpsimd if (qb > 0 and i % 32 < 4) else nc.vector
                        for hh in range(2):
                            eng.tensor_mul(
                                pt[:, hh * 512:(hh + 1) * 512],
                                ex[:, hh * 512:(hh + 1) * 512],
                                mq[:, mof:mof + 512],
                            )
                        pt_t[i] = pt
                    elif kind == "P":
                        i = op[1]
                        qb, ktile, pair = it_decode(i)
                        if i % 32 in (0, 1):
                            cq_t[(qb, pair)] = psum.tile(
                                [128, 1024], F32, name=f"cq{pair}",
                                tag="cqp", bufs=2)
                        cq = cq_t[(qb, pair)]
                        pt = pt_t.pop(i)
                        for hh in range(2):
                            h = pair * 2 + hh
                            nc.tensor.matmul(
                                cq[0:65, hh * 512:(hh + 1) * 512],
                                v_sb[ktile][:, h * 65:h * 65 + 65],
                                pt[:, hh * 512:(hh + 1) * 512],
                                start=(ktile == 0), stop=(ktile == NKT - 1),
                            )
                    elif kind == "VJ":
                        v_job(psum, op[1], "psa", 2)
                    elif kind == "KT":
                        p = op[1]
                        ps = psum.tile([128, 512], F32, name="psk", tag="psa", bufs=2)
                        for k in range(KC):
                            nc.tensor.matmul(
                                ps[:],
                                wsb[:, k * W3 + WK + p * 128:
                                    k * W3 + WK + (p + 1) * 128],
                                xk[k][:, 1536:2048],
                                start=(k == 0), stop=(k == KC - 1),
                            )
                        nc.vector.tensor_copy(kt_sb[p][:, 1536:2048], ps[:])
                    elif kind == "QJ":
                        q, p = op[1], op[2]
                        ps = psum.tile([128, 512], F32, name="psq", tag="psa", bufs=2)
                        for k in range(KC):
                            nc.tensor.matmul(
                                ps[:],
                                wsb[:, k * W3 + WQ + p * 128:
                                    k * W3 + WQ + (p + 1) * 128],
                                xk[k][:, q * 512:(q + 1) * 512],
                                start=(k == 0), stop=(k == KC - 1),
                            )
                        nc.vector.tensor_copy(qt[p][:, q * 512:(q + 1) * 512], ps[:])
                    elif kind == "CPc":
                        qb, c = op[1], op[2]
                        if c == 0:
                            _CACHE.setdefault("cqs_t", {})[qb] = work.tile(
                                [65, 2048], F32, name="cqs", tag="cqs", bufs=1)
                            rcb = work.tile([65, 2048], BF16, name="rcb",
                                            tag="rcb", bufs=1)
                            cn2 = work.tile([128, 1024], BF16, name="cn2",
                                            tag="cn2", bufs=1)
                            cno = work.tile([64, 1024], BF16, name="cno",
                                            tag="cno", bufs=1)
                            _CACHE.setdefault("rb_t", {})[qb] = (rcb, cn2, cno)
                        pair, hh = c // 2, c % 2
                        cqs = _CACHE["cqs_t"][qb]
                        cq = cq_t[(qb, pair)]
                        dst = cqs[:, c * 512:(c + 1) * 512]
                        src = cq[0:65, hh * 512:(hh + 1) * 512]
                        if qb < NQB - 1:
                            nc.vector.tensor_copy(dst, src)
                        else:
                            nc.scalar.copy(dst, src)
                        if c == 3:
                            cq_t.pop((qb, 0))
                            cq_t.pop((qb, 1))
                    elif kind == "R2":
                        qb, half = op[1], op[2]
                        rcb, cn2, cno = _CACHE["rb_t"][qb]
                        cqs = _CACHE["cqs_t"][qb]
                        sl = slice(half * 1024, (half + 1) * 1024)
                        nc.vector.tensor_copy(rcb[64:65, sl], cqs[64:65, sl])
                        rbp = psum.tile([64, 1024], F32, name="rbp", tag="cqp",
                                        bufs=2)
                        for c2 in range(2):
                            nc.tensor.matmul(
                                rbp[:, c2 * 512:(c2 + 1) * 512],
                                ones64[64:65, 0:64],
                                rcb[64:65, (half * 2 + c2) * 512:
                                    (half * 2 + c2 + 1) * 512],
                                start=True, stop=True,
                            )
                        rb32 = work.tile([64, 1024], F32, name="rb32",
                                         tag="rb32", bufs=2)
                        nc.vector.reciprocal_approx_fast(rb32[:], rbp[:])
                        _CACHE.setdefault("rbp_t", {})[(qb, half)] = rb32
                    elif kind == "CN":
                        qb, c = op[1], op[2]
                        rcb, cn2, cno = _CACHE["rb_t"][qb]
                        cqs = _CACHE["cqs_t"][qb]
                        rb32 = _CACHE["rbp_t"][(qb, c // 2)]
                        j = c // 2
                        src = cqs[0:64, c * 512:(c + 1) * 512]
                        rbc = rb32[:, (c % 2) * 512:(c % 2 + 1) * 512]
                        # CN on gpsimd: all-SBUF operands, keeps DVE free for
                        # the M-stream at boundaries
                        if c % 2 == 0:
                            nc.gpsimd.tensor_mul(
                                cn2[0:64, j * 512:(j + 1) * 512], src, rbc)
                        else:
                            nc.gpsimd.tensor_mul(
                                cno[:, j * 512:(j + 1) * 512], src, rbc)
                            nc.sync.dma_start(
                                cn2[64:128, j * 512:(j + 1) * 512],
                                cno[:, j * 512:(j + 1) * 512])
                        if c == 3:
                            cn_t[qb] = cn2
                            _CACHE["rb_t"].pop(qb)
                            _CACHE["cqs_t"].pop(qb)
                            _CACHE["rbp_t"].pop((qb, 0))
                            _CACHE["rbp_t"].pop((qb, 1))
                    elif kind == "O":
                        qb, g4 = op[1], op[2]
                        cn2 = cn_t[qb]
                        opp = psum.tile([128, 1024], F32, name="opp", tag="cqp",
                                        bufs=2)
                        for ot_l in range(2):
                            ot = 2 * g4 + ot_l
                            for j in range(2):
                                nc.tensor.matmul(
                                    opp[:, ot_l * 512:(ot_l + 1) * 512],
                                    wo_all[:, j * D + ot * 128:
                                           j * D + (ot + 1) * 128],
                                    cn2[:, j * 512:(j + 1) * 512],
                                    start=(j == 0), stop=(j == 1),
                                )
                        ysb = work.tile([128, 1024], BF16, name="ysb", tag="ysb", bufs=2)
                        if qb == NQB - 1 and g4 % 2 == 0:
                            nc.scalar.copy(ysb[:], opp[:])
                        else:
                            nc.vector.tensor_copy(ysb[:], opp[:])
                        nc.sync.dma_start(
                            yT_d[g4 * 256:(g4 + 1) * 256,
                                 qb * 512:(qb + 1) * 512].rearrange(
                                     "(o r) c -> r o c", o=2),
                            ysb.rearrange("r (o c) -> r o c", o=2),
                        )
                        if g4 == 3:
                            cn_t.pop(qb)
    nc.compile()
    return nc


def _get_nc():
    if "nc" not in _CACHE:
        _CACHE["nc"] = _build_nc()
    return _CACHE["nc"]


def kernel(x, mask, w_qkv, b_qkv, w_o, b_o):
    x = np.asarray(x, dtype=np.float32)
    mask = np.asarray(mask)
    w_qkv = np.asarray(w_qkv, dtype=np.float32)
    b_qkv = np.asarray(b_qkv, dtype=np.float32)
    w_o = np.asarray(w_o, dtype=np.float32)
    b_o = np.asarray(b_o, dtype=np.float32)
    assert not b_qkv.any(), "kernel specialized for zero qkv bias"

    scale = np.float32(1.0 / np.sqrt(HD))
    maskT = np.ascontiguousarray(mask.reshape(S, S).T).astype(
        ml_dtypes.bfloat16)

    w3 = w_qkv.reshape(H, 3, HD, D)  # [head, (q,k,v), hd, D]
    in_maps = []
    for c in range(N_CORES):
        b = c // 4
        h0 = (c % 4) * HL
        heads = list(range(h0, h0 + HL))
        wq = w3[heads, 0].reshape(CH, D) * scale
        wk = w3[heads, 1].reshape(CH, D)
        wv = w3[heads, 2].reshape(CH, D)
        # [K | Q | V] column packing
        wqkv = np.concatenate([wk.T, wq.T, wv.T], axis=1)  # [D, 3CH]
        wo_cols = np.concatenate([w_o[:, h * HD:(h + 1) * HD] for h in heads], axis=1)
        in_maps.append({
            "xT": np.ascontiguousarray(x[b].T).astype(ml_dtypes.bfloat16),
            "maskT": maskT,
            "wqkvT": np.ascontiguousarray(wqkv).astype(ml_dtypes.bfloat16),
            "woT": np.ascontiguousarray(wo_cols.T).astype(ml_dtypes.bfloat16),
        })

    nc = _get_nc()
    trace = bool(int(os.environ.get("MHA_TRACE", "0")))
    res = run_bass_kernel_spmd(nc, in_maps, core_ids=list(range(N_CORES)),
                               trace=trace)
    _CACHE["last_results"] = res

    y = np.zeros((B, S, D), dtype=np.float32)
    for c in range(N_CORES):
        y[c // 4] += np.asarray(res.results[c]["yT"], dtype=np.float32).T
    y += b_o
    return y
